# revision 5
# baseline (speedup 1.0000x reference)
"""Trainium2 Bass kernel for nn_AttentionHead (sparse causal+global attention).

Contract: kernel(**inputs) takes the FULL unsharded inputs
(q/k/v [8,2048,1024], Wq/Wk/Wv [128,1024], bq/bk/bv [128]) and returns
the FULL output [8,2048,128].

Sharding: data-parallel over batch -- one batch element per NeuronCore,
8 cores. Weights/masks replicated.

Device-side computation per core (batch element b), "transposed world":
  - host supplies xT = x[b].T  [1024, 2048] (c-major) for x in q,k,v
  - projections:  XT[d, s] = sum_c WxT[c,d]^T qT[c,s]  (+bias via ACT evict)
    giving d-major QT, KT, VT [128, 2048]; V is re-transposed on-chip
    (TensorE) to s-major blocks for the AV matmul.
  - scores^T tiles  St[sk=128, sq=512] = (KT block)^T @ (QT slice)  (PE, f32r)
  - P = exp(St / sqrt(128))  fused with PSUM eviction on ScalarE (no
    max-subtraction: |scores/sqrt(d)| <= ~2.5 for these inputs)
  - causal masking is STRUCTURAL: only sk-blocks i <= 4j+3 are computed for
    sq-tile j; diagonal blocks multiply by one of 4 static 0/1 patterns.
  - AV^T[d, sq] += V_block^T @ P   accumulated in PSUM over sk blocks
  - row sums via ones-vector matmul  [1,512] += ones^T @ P
  - global tokens (32 scattered rows+cols of the SxS mask) are handled by
    two narrow phases:
      B1: global KEYS for all queries (pairs sk in G, sk > sq)
      B2: global QUERIES vs non-global keys (pairs sq in G, sk > sq, sk not in G)
    Each phase outputs its own AV/sums; the host merges them (the
    active-pair sets of A/B1/B2 partition the reference mask exactly).
Host post-processing: out[b] = ((AVt + AVt_B1 [+scatter B2]) / sums).T
"""

import math
import os
import sys

import numpy as np

for _p in ("/opt/trn_rl_repo", "/root/.axon_site/_ro/trn_rl_repo"):
    if os.path.isdir(_p) and _p not in sys.path:
        sys.path.append(_p)

from contextlib import ExitStack

import concourse.bacc as bacc
import concourse.mybir as mybir
import concourse.tile as tile
from concourse.masks import make_identity

P = 128          # partitions / head dim
C = 1024         # input channels
G = 32           # number of global tokens
SQT = 512        # sq tile width (= max fp32 moving operand / PSUM bank)
NCH = C // P     # 8 contraction chunks for projections
B = 8            # batch / cores

F32 = mybir.dt.float32
F32R = mybir.dt.float32r
AFT = mybir.ActivationFunctionType


def _gtok(S):
    rng = np.random.default_rng(0)
    return rng.choice(S, size=G, replace=False)


def _host_masks(S):
    """Static 0/1 mask patterns, all tiny. float32."""
    gtok = _gtok(S)
    gset = np.zeros(S, dtype=bool)
    gset[gtok] = True
    nblk = S // P
    # 4 diagonal patterns: tile (sk_block i = 4j+t, sq_tile j):
    # active iff sq >= sk  <=>  f >= 128*t + p
    f = np.arange(SQT)[None, :]
    p = np.arange(P)[:, None]
    diag = np.stack(
        [(f >= P * t + p).astype(np.float32) for t in range(SQT // P)], axis=0
    )
    # B1: global keys, strictly above the diagonal: active iff gtok[g] > sq
    sq = np.arange(S)[None, :]
    mb1 = (gtok[:, None] > sq).astype(np.float32)  # [G, S]
    # B2: global queries vs non-global keys: active iff sk > gtok[g] and sk not in G
    sk = np.arange(S)[:, None]
    mb2 = ((sk > gtok[None, :]) & ~gset[:, None]).astype(np.float32)  # [S, G]
    mb2 = np.ascontiguousarray(mb2.reshape(nblk, P, G))
    return gtok, diag, mb1, mb2


def build_nc(S=2048, use_f32r=True):
    """Build the single-core Bass program (SPMD across 8 cores).

    float32r note: the walrus verifier requires every operand of an f32r
    matmul to live in an f32r-typed location (engines round on write), so
    all matmul-feeding DRAM inputs and SBUF tiles are declared float32r.
    Same 4-byte layout; numpy sees float32 on both ends.
    """
    nblk = S // P
    nj = S // SQT
    scale = 1.0 / math.sqrt(P)
    gtok = _gtok(S)
    DT = F32R if use_f32r else F32

    nc = bacc.Bacc("TRN2", target_bir_lowering=False, debug=False)

    def din(name, shape, dt=F32):
        return nc.dram_tensor(name, shape, dt, kind="ExternalInput").ap()

    def dout(name, shape):
        return nc.dram_tensor(name, shape, F32, kind="ExternalOutput").ap()

    qt_d = din("qt", [C, S], DT)
    kt_d = din("kt", [C, S], DT)
    vt_d = din("vt", [C, S], DT)
    wqt_d = din("wqt", [C, P], DT)
    wkt_d = din("wkt", [C, P], DT)
    wvt_d = din("wvt", [C, P], DT)
    bq_d = din("bq", [P, 1])
    bk_d = din("bk", [P, 1])
    bv_d = din("bv", [P, 1])
    diag_d = din("diag", [SQT // P, P, SQT], DT)
    mb1_d = din("mb1", [G, S], DT)
    mb2_d = din("mb2", [nblk, P, G], DT)
    ones_d = din("onesd", [P, 1], DT)

    avt_d = dout("avt", [P, S])
    sums_d = dout("sums", [1, S])
    avb1_d = dout("avb1", [P, S])
    sumsb1_d = dout("sumsb1", [1, S])
    avb2_d = dout("avb2", [P, G])
    sumsb2_d = dout("sumsb2", [1, G])

    def mmdt(ap):
        return ap

    with tile.TileContext(nc) as tc, ExitStack() as ctx:
        const = ctx.enter_context(tc.tile_pool(name="const", bufs=1))
        big = ctx.enter_context(tc.tile_pool(name="big", bufs=1))
        xin = ctx.enter_context(tc.tile_pool(name="xin", bufs=6))
        pp = ctx.enter_context(tc.tile_pool(name="pp", bufs=4))
        ev = ctx.enter_context(tc.tile_pool(name="ev", bufs=4))
        ps = ctx.enter_context(tc.tile_pool(name="ps", bufs=3, space="PSUM"))
        psav = ctx.enter_context(tc.tile_pool(name="psav", bufs=2, space="PSUM"))
        pssum = ctx.enter_context(tc.tile_pool(name="pssum", bufs=2, space="PSUM"))

        # ---- constants ----
        w_tiles = {}
        for nm, wd in (("q", wqt_d), ("k", wkt_d), ("v", wvt_d)):
            for c in range(NCH):
                t = const.tile([P, P], DT, name=f"w{nm}{c}", tag=f"w{nm}{c}")
                nc.sync.dma_start(t[:], wd[c * P : (c + 1) * P, :])
                w_tiles[nm, c] = t
        bias = {}
        for nm, bd in (("q", bq_d), ("k", bk_d), ("v", bv_d)):
            t = const.tile([P, 1], F32, name=f"b{nm}", tag=f"b{nm}")
            nc.sync.dma_start(t[:], bd[:])
            bias[nm] = t
        ones = const.tile([P, 1], DT, name="ones", tag="ones")
        nc.sync.dma_start(ones[:], ones_d[:])
        ident = const.tile([P, P], F32, name="ident", tag="ident")
        make_identity(nc, ident[:])
        diag_t = []
        for t_ in range(SQT // P):
            dt_ = const.tile([P, SQT], DT, name=f"diag{t_}", tag=f"diag{t_}")
            nc.sync.dma_start(dt_[:], diag_d[t_, :, :])
            diag_t.append(dt_)
        mb1_sb = const.tile([G, S], DT, name="mb1", tag="mb1")
        nc.sync.dma_start(mb1_sb[:], mb1_d[:])
        mb2_sb = []
        for i in range(nblk):
            t = const.tile([P, G], DT, name=f"mb2_{i}", tag=f"mb2_{i}")
            nc.sync.dma_start(t[:], mb2_d[i, :, :])
            mb2_sb.append(t)

        # ---- projected tensors (SBUF-resident) ----
        QT = big.tile([P, S], DT, name="QT", tag="QT")   # [d, sq]
        KT = big.tile([P, S], DT, name="KT", tag="KT")   # [d, sk]
        V = big.tile([P, S], DT, name="V", tag="V")      # 16 s-major blocks [sk,d]
        QG = big.tile([P, G], DT, name="QG", tag="QG")   # [d, g]
        KG = big.tile([P, G], DT, name="KG", tag="KG")   # [d, g]
        VG = big.tile([G, P], DT, name="VG", tag="VG")   # [g, d]

        # ---- phase 1: projections (d-major), interleaved by sq tile ----
        def project(nm, xd, j4, out_sb):
            psum = ps.tile([P, SQT], F32, name=f"pj{nm}{j4}", tag="ps")
            for c in range(NCH):
                xt = xin.tile([P, SQT], DT, name=f"x{nm}{j4}{c}", tag="xin")
                nc.sync.dma_start(
                    xt[:], xd[c * P : (c + 1) * P, j4 * SQT : (j4 + 1) * SQT]
                )
                nc.tensor.matmul(
                    psum[:],
                    lhsT=mmdt(w_tiles[nm, c][:]),
                    rhs=mmdt(xt[:]),
                    start=(c == 0),
                    stop=(c == NCH - 1),
                )
            # evict with per-partition bias add
            nc.scalar.activation(
                out_sb, psum[:], AFT.Identity, bias=bias[nm][:], scale=1.0
            )

        for j4 in range(nj):
            sl = slice(j4 * SQT, (j4 + 1) * SQT)
            project("q", qt_d, j4, QT[:, sl])
            project("k", kt_d, j4, KT[:, sl])
            vt_tmp = ev.tile([P, SQT], F32, name=f"vt{j4}", tag="ev")
            project("v", vt_d, j4, vt_tmp[:])
            # transpose VT (d-major) -> V (s-major blocks) via TensorE
            for t_ in range(SQT // P):
                blk = j4 * (SQT // P) + t_
                pst = ps.tile([P, P], F32, name=f"vtr{blk}", tag="ps")
                nc.tensor.matmul(
                    pst[:],
                    lhsT=vt_tmp[:, t_ * P : (t_ + 1) * P],
                    rhs=ident[:],
                    is_transpose=True,
                )
                nc.scalar.copy(V[:, blk * P : (blk + 1) * P], pst[:])

        # ---- gathers for global phases ----
        for g in range(G):
            tok = int(gtok[g])
            nc.vector.tensor_copy(QG[:, g : g + 1], QT[:, tok : tok + 1])
            nc.vector.tensor_copy(KG[:, g : g + 1], KT[:, tok : tok + 1])
            blk, p_ = tok // P, tok % P
            nc.gpsimd.dma_start(
                VG[g : g + 1, :], V[p_ : p_ + 1, blk * P : (blk + 1) * P]
            )

        # ---- phase 2: causal attention ----
        for j in range(nj):
            sl = slice(j * SQT, (j + 1) * SQT)
            av_ps = psav.tile([P, SQT], F32, name=f"av{j}", tag="psav")
            sm_ps = pssum.tile([1, SQT], F32, name=f"sm{j}", tag="pssum")
            nb = (j + 1) * (SQT // P)
            for i in range(nb):
                s_ps = ps.tile([P, SQT], F32, name=f"s{j}_{i}", tag="ps")
                nc.tensor.matmul(
                    s_ps[:],
                    lhsT=mmdt(KT[:, i * P : (i + 1) * P]),
                    rhs=mmdt(QT[:, sl]),
                    start=True,
                    stop=True,
                )
                p_sb = pp.tile([P, SQT], DT, name=f"p{j}_{i}", tag="pp")
                nc.scalar.activation(p_sb[:], s_ps[:], AFT.Exp, scale=scale)
                t_ = i - (SQT // P) * j
                if t_ >= 0:
                    nc.vector.tensor_mul(p_sb[:], p_sb[:], diag_t[t_][:])
                nc.tensor.matmul(
                    av_ps[:],
                    lhsT=mmdt(V[:, i * P : (i + 1) * P]),
                    rhs=mmdt(p_sb[:]),
                    start=(i == 0),
                    stop=(i == nb - 1),
                )
                nc.tensor.matmul(
                    sm_ps[:],
                    lhsT=mmdt(ones[:]),
                    rhs=mmdt(p_sb[:]),
                    start=(i == 0),
                    stop=(i == nb - 1),
                )
            av_sb = ev.tile([P, SQT], F32, name=f"avsb{j}", tag="ev")
            nc.scalar.copy(av_sb[:], av_ps[:])
            nc.sync.dma_start(avt_d[:, sl], av_sb[:])
            sm_sb = ev.tile([1, SQT], F32, name=f"smsb{j}", tag="evs")
            nc.scalar.copy(sm_sb[:], sm_ps[:])
            nc.sync.dma_start(sums_d[:, sl], sm_sb[:])

        # ---- phase B1: global keys (sk in G, sk > sq), all queries ----
        for j in range(nj):
            sl = slice(j * SQT, (j + 1) * SQT)
            s_ps = ps.tile([G, SQT], F32, name=f"b1s{j}", tag="ps")
            nc.tensor.matmul(
                s_ps[:], lhsT=mmdt(KG[:]), rhs=mmdt(QT[:, sl]), start=True, stop=True
            )
            p_sb = pp.tile([G, SQT], DT, name=f"b1p{j}", tag="pp")
            nc.scalar.activation(p_sb[:], s_ps[:], AFT.Exp, scale=scale)
            nc.vector.tensor_mul(p_sb[:], p_sb[:], mb1_sb[:, sl])
            av_ps = psav.tile([P, SQT], F32, name=f"b1av{j}", tag="psav")
            nc.tensor.matmul(
                av_ps[:], lhsT=mmdt(VG[:]), rhs=mmdt(p_sb[:]), start=True, stop=True
            )
            sm_ps = pssum.tile([1, SQT], F32, name=f"b1sm{j}", tag="pssum")
            nc.tensor.matmul(
                sm_ps[:],
                lhsT=mmdt(ones[0:G, :]),
                rhs=mmdt(p_sb[:]),
                start=True,
                stop=True,
            )
            av_sb = ev.tile([P, SQT], F32, name=f"b1avsb{j}", tag="ev")
            nc.scalar.copy(av_sb[:], av_ps[:])
            nc.sync.dma_start(avb1_d[:, sl], av_sb[:])
            sm_sb = ev.tile([1, SQT], F32, name=f"b1smsb{j}", tag="evs")
            nc.scalar.copy(sm_sb[:], sm_ps[:])
            nc.sync.dma_start(sumsb1_d[:, sl], sm_sb[:])

        # ---- phase B2: global queries vs non-global keys (sk > sq, sk not in G) ----
        av2_ps = psav.tile([P, G], F32, name="b2av", tag="psav")
        sm2_ps = pssum.tile([1, G], F32, name="b2sm", tag="pssum")
        for i in range(nblk):
            s_ps = ps.tile([P, G], F32, name=f"b2s{i}", tag="ps")
            nc.tensor.matmul(
                s_ps[:],
                lhsT=mmdt(KT[:, i * P : (i + 1) * P]),
                rhs=mmdt(QG[:]),
                start=True,
                stop=True,
            )
            p_sb = pp.tile([P, G], DT, name=f"b2p{i}", tag="pp")
            nc.scalar.activation(p_sb[:], s_ps[:], AFT.Exp, scale=scale)
            nc.vector.tensor_mul(p_sb[:], p_sb[:], mb2_sb[i][:])
            nc.tensor.matmul(
                av2_ps[:],
                lhsT=mmdt(V[:, i * P : (i + 1) * P]),
                rhs=mmdt(p_sb[:]),
                start=(i == 0),
                stop=(i == nblk - 1),
            )
            nc.tensor.matmul(
                sm2_ps[:],
                lhsT=mmdt(ones[:]),
                rhs=mmdt(p_sb[:]),
                start=(i == 0),
                stop=(i == nblk - 1),
            )
        av2_sb = ev.tile([P, G], F32, name="b2avsb", tag="ev")
        nc.scalar.copy(av2_sb[:], av2_ps[:])
        nc.sync.dma_start(avb2_d[:], av2_sb[:])
        sm2_sb = ev.tile([1, G], F32, name="b2smsb", tag="evs")
        nc.scalar.copy(sm2_sb[:], sm2_ps[:])
        nc.sync.dma_start(sumsb2_d[:], sm2_sb[:])

    nc.compile()
    return nc


def _in_maps(q, k, v, Wq, bq, Wk, bk, Wv, bv, S):
    gtok, diag, mb1, mb2 = _host_masks(S)
    shared = {
        "wqt": np.ascontiguousarray(Wq.T),
        "wkt": np.ascontiguousarray(Wk.T),
        "wvt": np.ascontiguousarray(Wv.T),
        "bq": np.ascontiguousarray(bq.reshape(P, 1)),
        "bk": np.ascontiguousarray(bk.reshape(P, 1)),
        "bv": np.ascontiguousarray(bv.reshape(P, 1)),
        "diag": diag,
        "mb1": mb1,
        "mb2": mb2,
        "onesd": np.ones((P, 1), dtype=np.float32),
    }
    maps = []
    for b in range(q.shape[0]):
        m = dict(shared)
        m["qt"] = np.ascontiguousarray(q[b].T)
        m["kt"] = np.ascontiguousarray(k[b].T)
        m["vt"] = np.ascontiguousarray(v[b].T)
        maps.append(m)
    return maps


def _assemble(results, S):
    gtok = _gtok(S)
    nb = len(results)
    out = np.empty((nb, S, P), dtype=np.float32)
    for b, r in enumerate(results):
        avt = r["avt"] + r["avb1"]
        sums = (r["sums"] + r["sumsb1"])[0]
        avt[:, gtok] += r["avb2"]
        sums[gtok] += r["sumsb2"][0]
        out[b] = (avt / sums[None, :]).T
    return out


_NC_CACHE = {}


def kernel(q, k, v, Wq, bq, Wk, bk, Wv, bv):
    from concourse.bass_utils import run_bass_kernel_spmd

    q = np.asarray(q, dtype=np.float32)
    k = np.asarray(k, dtype=np.float32)
    v = np.asarray(v, dtype=np.float32)
    S = q.shape[1]
    key = S
    if key not in _NC_CACHE:
        _NC_CACHE[key] = build_nc(S=S)
    nc = _NC_CACHE[key]
    maps = _in_maps(
        q, k, v,
        np.asarray(Wq, np.float32), np.asarray(bq, np.float32),
        np.asarray(Wk, np.float32), np.asarray(bk, np.float32),
        np.asarray(Wv, np.float32), np.asarray(bv, np.float32),
        S,
    )
    res = run_bass_kernel_spmd(nc, maps, core_ids=list(range(len(maps))))
    return _assemble(res.results, S)


# revision 7
# speedup vs baseline: 1.1059x; 1.1059x over previous
"""Trainium2 Bass kernel for nn_AttentionHead (sparse causal+global attention).

Contract: kernel(**inputs) takes the FULL unsharded inputs
(q/k/v [8,2048,1024], Wq/Wk/Wv [128,1024], bq/bk/bv [128]) and returns
the FULL output [8,2048,128].

Sharding: data-parallel over batch -- one batch element per NeuronCore,
8 cores. Weights/masks replicated.

Device-side computation per core (batch element b), "transposed world":
  - host supplies xT = x[b].T  [1024, 2048] (c-major) for x in q,k,v
  - projections:  XT[d, s] = sum_c WxT[c,d]^T xT[c,s]  (+bias on evict)
    giving d-major QT, KT, VT [128, 2048]; V is re-transposed on-chip
    (TensorE) to s-major blocks for the AV matmul.
  - scores^T tiles  St[sk=128, sq=512] = (KT block)^T @ (QT slice)  (PE, f32r)
  - P = exp(St / sqrt(128))  fused with PSUM eviction on ScalarE (no
    max-subtraction: |scores/sqrt(d)| <= ~2.5 for these inputs)
  - causal masking is STRUCTURAL: only sk-blocks i <= 4j+3 are computed for
    sq-tile j; diagonal blocks multiply by one of 4 static 0/1 patterns.
  - AV^T[d, sq] += V_block^T @ P   accumulated in PSUM over sk blocks
  - row sums via ones-vector matmul  [1,512] += ones^T @ P
  - global tokens (32 scattered rows+cols of the SxS mask) are handled by
    two narrow phases:
      B1: global KEYS for all queries (pairs sk in G, sk > sq)
      B2: global QUERIES vs non-global keys (pairs sq in G, sk > sq, sk not in G)
    Each phase outputs its own AV/sums; the host merges them (the
    active-pair sets of A/B1/B2 partition the reference mask exactly).
Host post-processing: out[b] = ((AVt + AVt_B1 [+scatter B2]) / sums).T

DMA notes: all small constants are packed host-side into one [128, 5633]
array (one fully-contiguous DMA) -- loading them individually serialized
~40us of descriptor-inefficient transfers ahead of the input stream.
Input chunks alternate between the two HWDGE rings (sync/SP and
scalar/ACT) to exceed the single-ring ~240-260 GB/s.

float32r: walrus requires every operand of an f32r matmul to live in an
f32r-typed location (engines round on write), so matmul-feeding DRAM
inputs and SBUF tiles are declared float32r. numpy sees float32.
"""

import math
import os
import sys

import numpy as np

for _p in ("/opt/trn_rl_repo", "/root/.axon_site/_ro/trn_rl_repo"):
    if os.path.isdir(_p) and _p not in sys.path:
        sys.path.append(_p)

from contextlib import ExitStack

import concourse.bacc as bacc
import concourse.mybir as mybir
import concourse.tile as tile
from concourse.masks import make_identity

P = 128          # partitions / head dim
C = 1024         # input channels
G = 32           # number of global tokens
SQT = 512        # sq tile width (= max fp32 moving operand / PSUM bank)
NCH = C // P     # 8 contraction chunks for projections
B = 8            # batch / cores

F32 = mybir.dt.float32
F32R = mybir.dt.float32r
AFT = mybir.ActivationFunctionType

# packed-constants column offsets
OFF_W = {"q": 0, "k": C, "v": 2 * C}
OFF_ONES = 3 * C
OFF_DIAG = 3 * C + 1
OFF_MB2 = 3 * C + 1 + 4 * SQT


def _cc_cols(S):
    return OFF_MB2 + (S // P) * G


def _gtok(S):
    rng = np.random.default_rng(0)
    return rng.choice(S, size=G, replace=False)


def _host_masks(S):
    """Static 0/1 mask patterns, all tiny. float32."""
    gtok = _gtok(S)
    gset = np.zeros(S, dtype=bool)
    gset[gtok] = True
    nblk = S // P
    # 4 diagonal patterns: tile (sk_block i = 4j+t, sq_tile j):
    # active iff sq >= sk  <=>  f >= 128*t + p
    f = np.arange(SQT)[None, :]
    p = np.arange(P)[:, None]
    diag = np.stack(
        [(f >= P * t + p).astype(np.float32) for t in range(SQT // P)], axis=0
    )
    # B1: global keys, strictly above the diagonal: active iff gtok[g] > sq
    sq = np.arange(S)[None, :]
    mb1 = (gtok[:, None] > sq).astype(np.float32)  # [G, S]
    # B2: global queries vs non-global keys: active iff sk > gtok[g], sk not in G
    sk = np.arange(S)[:, None]
    mb2 = ((sk > gtok[None, :]) & ~gset[:, None]).astype(np.float32)  # [S, G]
    mb2 = np.ascontiguousarray(mb2.reshape(nblk, P, G))
    return gtok, diag, mb1, mb2


def _pack_consts(Wq, Wk, Wv, S):
    """One [128, CC_COLS] array: per-partition-contiguous packing of the
    projection weight chunks, ones column, diag patterns and mb2."""
    _, diag, _, mb2 = _host_masks(S)
    nblk = S // P

    def wpack(W):
        wt = np.ascontiguousarray(W.T)            # [C, P] = WxT
        return np.ascontiguousarray(
            wt.reshape(NCH, P, P).transpose(1, 0, 2).reshape(P, C)
        )

    cc = np.empty((P, _cc_cols(S)), dtype=np.float32)
    cc[:, OFF_W["q"] : OFF_W["q"] + C] = wpack(Wq)
    cc[:, OFF_W["k"] : OFF_W["k"] + C] = wpack(Wk)
    cc[:, OFF_W["v"] : OFF_W["v"] + C] = wpack(Wv)
    cc[:, OFF_ONES] = 1.0
    cc[:, OFF_DIAG : OFF_DIAG + 4 * SQT] = diag.transpose(1, 0, 2).reshape(P, 4 * SQT)
    cc[:, OFF_MB2 : OFF_MB2 + nblk * G] = mb2.transpose(1, 0, 2).reshape(P, nblk * G)
    return cc


def build_nc(S=2048, use_f32r=True):
    """Build the single-core Bass program (SPMD across 8 cores)."""
    nblk = S // P
    nj = S // SQT
    scale = 1.0 / math.sqrt(P)
    gtok = _gtok(S)
    DT = F32R if use_f32r else F32

    nc = bacc.Bacc("TRN2", target_bir_lowering=False, debug=False)

    def din(name, shape, dt=F32):
        return nc.dram_tensor(name, shape, dt, kind="ExternalInput").ap()

    def dout(name, shape):
        return nc.dram_tensor(name, shape, F32, kind="ExternalOutput").ap()

    qt_d = din("qt", [C, S], DT)
    kt_d = din("kt", [C, S], DT)
    vt_d = din("vt", [C, S], DT)
    cc_d = din("cc", [P, _cc_cols(S)], DT)
    bias_d = din("biases", [P, 3])
    mb1_d = din("mb1", [G, S], DT)

    avt_d = dout("avt", [P, S])
    sums_d = dout("sums", [1, S])
    avb1_d = dout("avb1", [P, S])
    sumsb1_d = dout("sumsb1", [1, S])
    avb2_d = dout("avb2", [P, G])
    sumsb2_d = dout("sumsb2", [1, G])

    # round-robin between the two HWDGE rings for the big input stream
    rings = [nc.sync, nc.scalar]
    ring_i = [0]

    def ring():
        ring_i[0] ^= 1
        return rings[ring_i[0]]

    with tile.TileContext(nc) as tc, ExitStack() as ctx:
        const = ctx.enter_context(tc.tile_pool(name="const", bufs=1))
        big = ctx.enter_context(tc.tile_pool(name="big", bufs=1))
        xin = ctx.enter_context(tc.tile_pool(name="xin", bufs=6))
        pp = ctx.enter_context(tc.tile_pool(name="pp", bufs=4))
        ev = ctx.enter_context(tc.tile_pool(name="ev", bufs=4))
        ps = ctx.enter_context(tc.tile_pool(name="ps", bufs=3, space="PSUM"))
        psav = ctx.enter_context(tc.tile_pool(name="psav", bufs=2, space="PSUM"))
        pssum = ctx.enter_context(tc.tile_pool(name="pssum", bufs=2, space="PSUM"))

        # ---- constants: one packed DMA + biases + mb1 ----
        CCt = const.tile([P, _cc_cols(S)], DT, name="CC", tag="CC")
        nc.sync.dma_start(CCt[:], cc_d[:])
        bias_sb = const.tile([P, 3], F32, name="biases", tag="biases")
        nc.sync.dma_start(bias_sb[:], bias_d[:])
        mb1_sb = const.tile([G, S], DT, name="mb1", tag="mb1")
        nc.sync.dma_start(mb1_sb[:], mb1_d[:])
        ident = const.tile([P, P], F32, name="ident", tag="ident")
        make_identity(nc, ident[:])

        def wtile(nm, c):
            return CCt[:, OFF_W[nm] + c * P : OFF_W[nm] + (c + 1) * P]

        ones = CCt[:, OFF_ONES : OFF_ONES + 1]
        bias = {
            "q": bias_sb[:, 0:1],
            "k": bias_sb[:, 1:2],
            "v": bias_sb[:, 2:3],
        }

        def diag_t(t_):
            return CCt[:, OFF_DIAG + t_ * SQT : OFF_DIAG + (t_ + 1) * SQT]

        def mb2_t(i):
            return CCt[:, OFF_MB2 + i * G : OFF_MB2 + (i + 1) * G]

        # ---- projected tensors (SBUF-resident) ----
        QT = big.tile([P, S], DT, name="QT", tag="QT")   # [d, sq]
        KT = big.tile([P, S], DT, name="KT", tag="KT")   # [d, sk]
        V = big.tile([P, S], DT, name="V", tag="V")      # 16 s-major blocks [sk,d]
        QG = big.tile([P, G], DT, name="QG", tag="QG")   # [d, g]
        KG = big.tile([P, G], DT, name="KG", tag="KG")   # [d, g]
        VG = big.tile([G, P], DT, name="VG", tag="VG")   # [g, d]

        # ---- phase 1: projections (d-major), interleaved by sq tile ----
        def project(nm, xd, j4, out_sb, out_f32=False):
            psum = ps.tile([P, SQT], F32, name=f"pj{nm}{j4}", tag="ps")
            for c in range(NCH):
                xt = xin.tile([P, SQT], DT, name=f"x{nm}{j4}{c}", tag="xin")
                ring().dma_start(
                    xt[:], xd[c * P : (c + 1) * P, j4 * SQT : (j4 + 1) * SQT]
                )
                nc.tensor.matmul(
                    psum[:],
                    lhsT=wtile(nm, c),
                    rhs=xt[:],
                    start=(c == 0),
                    stop=(c == NCH - 1),
                )
            # evict with per-partition bias add (on DVE; ACT is kept for exp)
            nc.vector.tensor_scalar_add(out_sb, psum[:], bias[nm])

        for j4 in range(nj):
            sl = slice(j4 * SQT, (j4 + 1) * SQT)
            project("q", qt_d, j4, QT[:, sl])
            project("k", kt_d, j4, KT[:, sl])
            vt_tmp = ev.tile([P, SQT], F32, name=f"vt{j4}", tag="ev")
            project("v", vt_d, j4, vt_tmp[:])
            # transpose VT (d-major) -> V (s-major blocks) via TensorE
            for t_ in range(SQT // P):
                blk = j4 * (SQT // P) + t_
                pst = ps.tile([P, P], F32, name=f"vtr{blk}", tag="ps")
                nc.tensor.matmul(
                    pst[:],
                    lhsT=vt_tmp[:, t_ * P : (t_ + 1) * P],
                    rhs=ident[:],
                    is_transpose=True,
                )
                nc.vector.tensor_copy(V[:, blk * P : (blk + 1) * P], pst[:])

        # ---- gathers for global phases ----
        for g in range(G):
            tok = int(gtok[g])
            nc.vector.tensor_copy(QG[:, g : g + 1], QT[:, tok : tok + 1])
            nc.vector.tensor_copy(KG[:, g : g + 1], KT[:, tok : tok + 1])
            blk, p_ = tok // P, tok % P
            nc.gpsimd.dma_start(
                VG[g : g + 1, :], V[p_ : p_ + 1, blk * P : (blk + 1) * P]
            )

        # ---- phase 2: causal attention ----
        for j in range(nj):
            sl = slice(j * SQT, (j + 1) * SQT)
            av_ps = psav.tile([P, SQT], F32, name=f"av{j}", tag="psav")
            sm_ps = pssum.tile([1, SQT], F32, name=f"sm{j}", tag="pssum")
            nb = (j + 1) * (SQT // P)
            for i in range(nb):
                s_ps = ps.tile([P, SQT], F32, name=f"s{j}_{i}", tag="ps")
                nc.tensor.matmul(
                    s_ps[:],
                    lhsT=KT[:, i * P : (i + 1) * P],
                    rhs=QT[:, sl],
                    start=True,
                    stop=True,
                )
                p_sb = pp.tile([P, SQT], DT, name=f"p{j}_{i}", tag="pp")
                nc.scalar.activation(p_sb[:], s_ps[:], AFT.Exp, scale=scale)
                t_ = i - (SQT // P) * j
                if t_ >= 0:
                    nc.vector.tensor_mul(p_sb[:], p_sb[:], diag_t(t_))
                nc.tensor.matmul(
                    av_ps[:],
                    lhsT=V[:, i * P : (i + 1) * P],
                    rhs=p_sb[:],
                    start=(i == 0),
                    stop=(i == nb - 1),
                )
                nc.tensor.matmul(
                    sm_ps[:],
                    lhsT=ones,
                    rhs=p_sb[:],
                    start=(i == 0),
                    stop=(i == nb - 1),
                )
            av_sb = ev.tile([P, SQT], F32, name=f"avsb{j}", tag="ev")
            nc.vector.tensor_copy(av_sb[:], av_ps[:])
            nc.scalar.dma_start(avt_d[:, sl], av_sb[:])
            sm_sb = ev.tile([1, SQT], F32, name=f"smsb{j}", tag="evs")
            nc.vector.tensor_copy(sm_sb[:], sm_ps[:])
            nc.scalar.dma_start(sums_d[:, sl], sm_sb[:])

        # ---- phase B1: global keys (sk in G, sk > sq), all queries ----
        for j in range(nj):
            sl = slice(j * SQT, (j + 1) * SQT)
            s_ps = ps.tile([G, SQT], F32, name=f"b1s{j}", tag="ps")
            nc.tensor.matmul(
                s_ps[:], lhsT=KG[:], rhs=QT[:, sl], start=True, stop=True
            )
            p_sb = pp.tile([G, SQT], DT, name=f"b1p{j}", tag="pp")
            nc.scalar.activation(p_sb[:], s_ps[:], AFT.Exp, scale=scale)
            nc.vector.tensor_mul(p_sb[:], p_sb[:], mb1_sb[:, sl])
            av_ps = psav.tile([P, SQT], F32, name=f"b1av{j}", tag="psav")
            nc.tensor.matmul(
                av_ps[:], lhsT=VG[:], rhs=p_sb[:], start=True, stop=True
            )
            sm_ps = pssum.tile([1, SQT], F32, name=f"b1sm{j}", tag="pssum")
            nc.tensor.matmul(
                sm_ps[:], lhsT=ones[0:G, :], rhs=p_sb[:], start=True, stop=True
            )
            av_sb = ev.tile([P, SQT], F32, name=f"b1avsb{j}", tag="ev")
            nc.vector.tensor_copy(av_sb[:], av_ps[:])
            nc.scalar.dma_start(avb1_d[:, sl], av_sb[:])
            sm_sb = ev.tile([1, SQT], F32, name=f"b1smsb{j}", tag="evs")
            nc.vector.tensor_copy(sm_sb[:], sm_ps[:])
            nc.scalar.dma_start(sumsb1_d[:, sl], sm_sb[:])

        # ---- phase B2: global queries vs non-global keys ----
        av2_ps = psav.tile([P, G], F32, name="b2av", tag="psav")
        sm2_ps = pssum.tile([1, G], F32, name="b2sm", tag="pssum")
        for i in range(nblk):
            s_ps = ps.tile([P, G], F32, name=f"b2s{i}", tag="ps")
            nc.tensor.matmul(
                s_ps[:],
                lhsT=KT[:, i * P : (i + 1) * P],
                rhs=QG[:],
                start=True,
                stop=True,
            )
            p_sb = pp.tile([P, G], DT, name=f"b2p{i}", tag="pp")
            nc.scalar.activation(p_sb[:], s_ps[:], AFT.Exp, scale=scale)
            nc.vector.tensor_mul(p_sb[:], p_sb[:], mb2_t(i))
            nc.tensor.matmul(
                av2_ps[:],
                lhsT=V[:, i * P : (i + 1) * P],
                rhs=p_sb[:],
                start=(i == 0),
                stop=(i == nblk - 1),
            )
            nc.tensor.matmul(
                sm2_ps[:],
                lhsT=ones,
                rhs=p_sb[:],
                start=(i == 0),
                stop=(i == nblk - 1),
            )
        av2_sb = ev.tile([P, G], F32, name="b2avsb", tag="ev")
        nc.vector.tensor_copy(av2_sb[:], av2_ps[:])
        nc.scalar.dma_start(avb2_d[:], av2_sb[:])
        sm2_sb = ev.tile([1, G], F32, name="b2smsb", tag="evs")
        nc.vector.tensor_copy(sm2_sb[:], sm2_ps[:])
        nc.scalar.dma_start(sumsb2_d[:], sm2_sb[:])

    nc.compile()
    return nc


def _in_maps(q, k, v, Wq, bq, Wk, bk, Wv, bv, S):
    _, _, mb1, _ = _host_masks(S)
    shared = {
        "cc": _pack_consts(Wq, Wk, Wv, S),
        "biases": np.ascontiguousarray(
            np.stack([bq, bk, bv], axis=1).astype(np.float32)
        ),
        "mb1": mb1,
    }
    maps = []
    for b in range(q.shape[0]):
        m = dict(shared)
        m["qt"] = np.ascontiguousarray(q[b].T)
        m["kt"] = np.ascontiguousarray(k[b].T)
        m["vt"] = np.ascontiguousarray(v[b].T)
        maps.append(m)
    return maps


def _assemble(results, S):
    gtok = _gtok(S)
    nb = len(results)
    out = np.empty((nb, S, P), dtype=np.float32)
    for b, r in enumerate(results):
        avt = r["avt"] + r["avb1"]
        sums = (r["sums"] + r["sumsb1"])[0]
        avt[:, gtok] += r["avb2"]
        sums[gtok] += r["sumsb2"][0]
        out[b] = (avt / sums[None, :]).T
    return out


_NC_CACHE = {}


def kernel(q, k, v, Wq, bq, Wk, bk, Wv, bv):
    from concourse.bass_utils import run_bass_kernel_spmd

    q = np.asarray(q, dtype=np.float32)
    k = np.asarray(k, dtype=np.float32)
    v = np.asarray(v, dtype=np.float32)
    S = q.shape[1]
    if S not in _NC_CACHE:
        _NC_CACHE[S] = build_nc(S=S)
    nc = _NC_CACHE[S]
    maps = _in_maps(
        q, k, v,
        np.asarray(Wq, np.float32), np.asarray(bq, np.float32),
        np.asarray(Wk, np.float32), np.asarray(bk, np.float32),
        np.asarray(Wv, np.float32), np.asarray(bv, np.float32),
        S,
    )
    res = run_bass_kernel_spmd(nc, maps, core_ids=list(range(len(maps))))
    return _assemble(res.results, S)


# revision 8
# speedup vs baseline: 1.3406x; 1.2122x over previous
"""Trainium2 Bass kernel for nn_AttentionHead (sparse causal+global attention).

Contract: kernel(**inputs) takes the FULL unsharded inputs
(q/k/v [8,2048,1024], Wq/Wk/Wv [128,1024], bq/bk/bv [128]) and returns
the FULL output [8,2048,128].

Sharding: data-parallel over batch -- one batch element per NeuronCore,
8 cores. Weights/masks replicated.

Device-side computation per core (batch element b), "transposed world":
  - host supplies xT = x[b].T  [1024, 2048] (c-major) for x in q,k,v
  - projections:  XT[d, s] = sum_c WxT[c,d]^T xT[c,s]  (+bias on evict)
    giving d-major QT, KT, VT [128, 2048]; V is re-transposed on-chip
    (TensorE) to s-major blocks for the AV matmul.
  - scores^T tiles  St[sk=128, sq=512] = (KT block)^T @ (QT slice)  (PE, f32r)
  - P = exp(St / sqrt(128))  fused with PSUM eviction on ScalarE (no
    max-subtraction: |scores/sqrt(d)| <= ~2.5 for these inputs)
  - causal masking is STRUCTURAL: only sk-blocks i <= 4j+3 are computed for
    sq-tile j; diagonal blocks multiply by one of 4 static 0/1 patterns.
  - AV^T[d, sq] += V_block^T @ P   accumulated in PSUM over sk blocks
  - row sums via ones-vector matmul  [1,512] += ones^T @ P
  - global tokens (32 scattered rows+cols of the SxS mask) are handled by
    two narrow phases:
      B1: global KEYS for all queries (pairs sk in G, sk > sq)
      B2: global QUERIES vs non-global keys (pairs sq in G, sk > sq, sk not in G)
    Each phase outputs its own AV/sums; the host merges them (the
    active-pair sets of A/B1/B2 partition the reference mask exactly).
Host post-processing: out[b] = ((AVt + AVt_B1 [+scatter B2]) / sums).T

DMA notes: all small constants are packed host-side into one [128, 5633]
array (one fully-contiguous DMA) -- loading them individually serialized
~40us of descriptor-inefficient transfers ahead of the input stream.
Input chunks alternate between the two HWDGE rings (sync/SP and
scalar/ACT) to exceed the single-ring ~240-260 GB/s.

float32r: walrus requires every operand of an f32r matmul to live in an
f32r-typed location (engines round on write), so matmul-feeding DRAM
inputs and SBUF tiles are declared float32r. numpy sees float32.
"""

import math
import os
import sys

import numpy as np

for _p in ("/opt/trn_rl_repo", "/root/.axon_site/_ro/trn_rl_repo"):
    if os.path.isdir(_p) and _p not in sys.path:
        sys.path.append(_p)

from contextlib import ExitStack

import concourse.bacc as bacc
import concourse.mybir as mybir
import concourse.tile as tile
from concourse.masks import make_identity

P = 128          # partitions / head dim
C = 1024         # input channels
G = 32           # number of global tokens
SQT = 512        # sq tile width (= max fp32 moving operand / PSUM bank)
NCH = C // P     # 8 contraction chunks for projections
B = 8            # batch / cores

F32 = mybir.dt.float32
F32R = mybir.dt.float32r
AFT = mybir.ActivationFunctionType

# packed-constants column offsets
OFF_W = {"q": 0, "k": C, "v": 2 * C}
OFF_ONES = 3 * C
OFF_DIAG = 3 * C + 1
OFF_MB2 = 3 * C + 1 + 4 * SQT


def _cc_cols(S):
    return OFF_MB2 + (S // P) * G


def _gtok(S):
    rng = np.random.default_rng(0)
    return rng.choice(S, size=G, replace=False)


def _host_masks(S):
    """Static 0/1 mask patterns, all tiny. float32."""
    gtok = _gtok(S)
    gset = np.zeros(S, dtype=bool)
    gset[gtok] = True
    nblk = S // P
    # 4 diagonal patterns: tile (sk_block i = 4j+t, sq_tile j):
    # active iff sq >= sk  <=>  f >= 128*t + p
    f = np.arange(SQT)[None, :]
    p = np.arange(P)[:, None]
    diag = np.stack(
        [(f >= P * t + p).astype(np.float32) for t in range(SQT // P)], axis=0
    )
    # B1: global keys, strictly above the diagonal: active iff gtok[g] > sq
    sq = np.arange(S)[None, :]
    mb1 = (gtok[:, None] > sq).astype(np.float32)  # [G, S]
    # B2: global queries vs non-global keys: active iff sk > gtok[g], sk not in G
    sk = np.arange(S)[:, None]
    mb2 = ((sk > gtok[None, :]) & ~gset[:, None]).astype(np.float32)  # [S, G]
    mb2 = np.ascontiguousarray(mb2.reshape(nblk, P, G))
    return gtok, diag, mb1, mb2


def _pack_consts(Wq, Wk, Wv, S):
    """One [128, CC_COLS] array: per-partition-contiguous packing of the
    projection weight chunks, ones column, diag patterns and mb2."""
    _, diag, _, mb2 = _host_masks(S)
    nblk = S // P

    def wpack(W):
        wt = np.ascontiguousarray(W.T)            # [C, P] = WxT
        return np.ascontiguousarray(
            wt.reshape(NCH, P, P).transpose(1, 0, 2).reshape(P, C)
        )

    cc = np.empty((P, _cc_cols(S)), dtype=np.float32)
    cc[:, OFF_W["q"] : OFF_W["q"] + C] = wpack(Wq)
    cc[:, OFF_W["k"] : OFF_W["k"] + C] = wpack(Wk)
    cc[:, OFF_W["v"] : OFF_W["v"] + C] = wpack(Wv)
    cc[:, OFF_ONES] = 1.0
    cc[:, OFF_DIAG : OFF_DIAG + 4 * SQT] = diag.transpose(1, 0, 2).reshape(P, 4 * SQT)
    cc[:, OFF_MB2 : OFF_MB2 + nblk * G] = mb2.transpose(1, 0, 2).reshape(P, nblk * G)
    return cc


def build_nc(S=2048, use_f32r=True):
    """Build the single-core Bass program (SPMD across 8 cores)."""
    nblk = S // P
    nj = S // SQT
    scale = 1.0 / math.sqrt(P)
    gtok = _gtok(S)
    DT = F32R if use_f32r else F32

    nc = bacc.Bacc("TRN2", target_bir_lowering=False, debug=False)

    def din(name, shape, dt=F32):
        return nc.dram_tensor(name, shape, dt, kind="ExternalInput").ap()

    def dout(name, shape):
        return nc.dram_tensor(name, shape, F32, kind="ExternalOutput").ap()

    qt_d = din("qt", [C, S], DT)
    kt_d = din("kt", [C, S], DT)
    vt_d = din("vt", [C, S], DT)
    cc_d = din("cc", [P, _cc_cols(S)], DT)
    bias_d = din("biases", [P, 3])
    mb1_d = din("mb1", [G, S], DT)

    avt_d = dout("avt", [P, S])
    sums_d = dout("sums", [1, S])
    avb1_d = dout("avb1", [P, S])
    sumsb1_d = dout("sumsb1", [1, S])
    avb2_d = dout("avb2", [P, G])
    sumsb2_d = dout("sumsb2", [1, G])

    # round-robin the big input stream between the SP HWDGE ring and the
    # GPSIMD SWDGE queue (ScalarE must stay free for exp; DMA issues on it
    # head-of-line block the activation stream)
    rings = [nc.sync, nc.gpsimd]
    ring_i = [0]

    def ring():
        ring_i[0] ^= 1
        return rings[ring_i[0]]

    with tile.TileContext(nc) as tc, ExitStack() as ctx:
        const = ctx.enter_context(tc.tile_pool(name="const", bufs=1))
        big = ctx.enter_context(tc.tile_pool(name="big", bufs=1))
        xin = ctx.enter_context(tc.tile_pool(name="xin", bufs=6))
        pp = ctx.enter_context(tc.tile_pool(name="pp", bufs=4))
        ev = ctx.enter_context(tc.tile_pool(name="ev", bufs=4))
        ps = ctx.enter_context(tc.tile_pool(name="ps", bufs=4, space="PSUM"))
        psav = ctx.enter_context(tc.tile_pool(name="psav", bufs=2, space="PSUM"))
        pssum = ctx.enter_context(tc.tile_pool(name="pssum", bufs=2, space="PSUM"))

        # ---- constants: one packed DMA + biases + mb1 ----
        CCt = const.tile([P, _cc_cols(S)], DT, name="CC", tag="CC")
        nc.sync.dma_start(CCt[:], cc_d[:])
        bias_sb = const.tile([P, 3], F32, name="biases", tag="biases")
        nc.sync.dma_start(bias_sb[:], bias_d[:])
        mb1_sb = const.tile([G, S], DT, name="mb1", tag="mb1")
        nc.sync.dma_start(mb1_sb[:], mb1_d[:])
        ident = const.tile([P, P], F32, name="ident", tag="ident")
        make_identity(nc, ident[:])

        def wtile(nm, c):
            return CCt[:, OFF_W[nm] + c * P : OFF_W[nm] + (c + 1) * P]

        ones = CCt[:, OFF_ONES : OFF_ONES + 1]
        bias = {
            "q": bias_sb[:, 0:1],
            "k": bias_sb[:, 1:2],
            "v": bias_sb[:, 2:3],
        }

        def diag_t(t_):
            return CCt[:, OFF_DIAG + t_ * SQT : OFF_DIAG + (t_ + 1) * SQT]

        def mb2_t(i):
            return CCt[:, OFF_MB2 + i * G : OFF_MB2 + (i + 1) * G]

        # ---- projected tensors (SBUF-resident) ----
        QT = big.tile([P, S], DT, name="QT", tag="QT")   # [d, sq]
        KT = big.tile([P, S], DT, name="KT", tag="KT")   # [d, sk]
        V = big.tile([P, S], DT, name="V", tag="V")      # 16 s-major blocks [sk,d]
        QG = big.tile([P, G], DT, name="QG", tag="QG")   # [d, g]
        KG = big.tile([P, G], DT, name="KG", tag="KG")   # [d, g]
        VG = big.tile([G, P], DT, name="VG", tag="VG")   # [g, d]

        # ---- phase 1: projections (d-major) in [128, 1024] chunks (4KB
        # contiguous lines), two PSUM halves per chunk; attention sq-tiles
        # are emitted as soon as their projections exist so PE/DMA overlap
        CW = 2 * SQT  # dma chunk width

        def project(nm, xd, j2, out_a, out_b):
            psA = ps.tile([P, SQT], F32, name=f"pj{nm}{j2}a", tag="ps")
            psB = ps.tile([P, SQT], F32, name=f"pj{nm}{j2}b", tag="ps")
            for c in range(NCH):
                xt = xin.tile([P, CW], DT, name=f"x{nm}{j2}{c}", tag="xin")
                ring().dma_start(
                    xt[:], xd[c * P : (c + 1) * P, j2 * CW : (j2 + 1) * CW]
                )
                nc.tensor.matmul(
                    psA[:], lhsT=wtile(nm, c), rhs=xt[:, 0:SQT],
                    start=(c == 0), stop=(c == NCH - 1),
                )
                nc.tensor.matmul(
                    psB[:], lhsT=wtile(nm, c), rhs=xt[:, SQT:CW],
                    start=(c == 0), stop=(c == NCH - 1),
                )
            # evict with per-partition bias add (on DVE; ACT is kept for exp)
            nc.vector.tensor_scalar_add(out_a, psA[:], bias[nm])
            nc.vector.tensor_scalar_add(out_b, psB[:], bias[nm])

        def attention_j(j):
            sl = slice(j * SQT, (j + 1) * SQT)
            av_ps = psav.tile([P, SQT], F32, name=f"av{j}", tag="psav")
            sm_ps = pssum.tile([1, SQT], F32, name=f"sm{j}", tag="pssum")
            nb = (j + 1) * (SQT // P)
            for i in range(nb):
                s_ps = ps.tile([P, SQT], F32, name=f"s{j}_{i}", tag="ps")
                nc.tensor.matmul(
                    s_ps[:],
                    lhsT=KT[:, i * P : (i + 1) * P],
                    rhs=QT[:, sl],
                    start=True,
                    stop=True,
                )
                p_sb = pp.tile([P, SQT], DT, name=f"p{j}_{i}", tag="pp")
                nc.scalar.activation(p_sb[:], s_ps[:], AFT.Exp, scale=scale)
                t_ = i - (SQT // P) * j
                if t_ >= 0:
                    nc.vector.tensor_mul(p_sb[:], p_sb[:], diag_t(t_))
                nc.tensor.matmul(
                    av_ps[:],
                    lhsT=V[:, i * P : (i + 1) * P],
                    rhs=p_sb[:],
                    start=(i == 0),
                    stop=(i == nb - 1),
                )
                nc.tensor.matmul(
                    sm_ps[:],
                    lhsT=ones,
                    rhs=p_sb[:],
                    start=(i == 0),
                    stop=(i == nb - 1),
                )
            av_sb = ev.tile([P, SQT], F32, name=f"avsb{j}", tag="ev")
            nc.vector.tensor_copy(av_sb[:], av_ps[:])
            nc.sync.dma_start(avt_d[:, sl], av_sb[:])
            sm_sb = ev.tile([1, SQT], F32, name=f"smsb{j}", tag="evs")
            nc.vector.tensor_copy(sm_sb[:], sm_ps[:])
            nc.sync.dma_start(sums_d[:, sl], sm_sb[:])

        for j2 in range(nj // 2):
            sla = slice(j2 * CW, j2 * CW + SQT)
            slb = slice(j2 * CW + SQT, (j2 + 1) * CW)
            project("q", qt_d, j2, QT[:, sla], QT[:, slb])
            project("k", kt_d, j2, KT[:, sla], KT[:, slb])
            vt_a = ev.tile([P, SQT], F32, name=f"vta{j2}", tag="ev")
            vt_b = ev.tile([P, SQT], F32, name=f"vtb{j2}", tag="ev")
            project("v", vt_d, j2, vt_a[:], vt_b[:])
            # transpose VT (d-major) -> V (s-major blocks) via TensorE
            for h, vt_tmp in ((0, vt_a), (1, vt_b)):
                for t_ in range(SQT // P):
                    blk = (2 * j2 + h) * (SQT // P) + t_
                    pst = ps.tile([P, P], F32, name=f"vtr{blk}", tag="ps")
                    nc.tensor.matmul(
                        pst[:],
                        lhsT=vt_tmp[:, t_ * P : (t_ + 1) * P],
                        rhs=ident[:],
                        is_transpose=True,
                    )
                    nc.vector.tensor_copy(V[:, blk * P : (blk + 1) * P], pst[:])
            attention_j(2 * j2)
            attention_j(2 * j2 + 1)

        # ---- gathers for global phases (only B1/B2 depend on these) ----
        for g in range(G):
            tok = int(gtok[g])
            nc.vector.tensor_copy(QG[:, g : g + 1], QT[:, tok : tok + 1])
            nc.vector.tensor_copy(KG[:, g : g + 1], KT[:, tok : tok + 1])
            blk, p_ = tok // P, tok % P
            nc.gpsimd.dma_start(
                VG[g : g + 1, :], V[p_ : p_ + 1, blk * P : (blk + 1) * P]
            )

        # ---- phase B1: global keys (sk in G, sk > sq), all queries ----
        for j in range(nj):
            sl = slice(j * SQT, (j + 1) * SQT)
            s_ps = ps.tile([G, SQT], F32, name=f"b1s{j}", tag="ps")
            nc.tensor.matmul(
                s_ps[:], lhsT=KG[:], rhs=QT[:, sl], start=True, stop=True
            )
            p_sb = pp.tile([G, SQT], DT, name=f"b1p{j}", tag="pp")
            nc.scalar.activation(p_sb[:], s_ps[:], AFT.Exp, scale=scale)
            nc.vector.tensor_mul(p_sb[:], p_sb[:], mb1_sb[:, sl])
            av_ps = psav.tile([P, SQT], F32, name=f"b1av{j}", tag="psav")
            nc.tensor.matmul(
                av_ps[:], lhsT=VG[:], rhs=p_sb[:], start=True, stop=True
            )
            sm_ps = pssum.tile([1, SQT], F32, name=f"b1sm{j}", tag="pssum")
            nc.tensor.matmul(
                sm_ps[:], lhsT=ones[0:G, :], rhs=p_sb[:], start=True, stop=True
            )
            av_sb = ev.tile([P, SQT], F32, name=f"b1avsb{j}", tag="ev")
            nc.vector.tensor_copy(av_sb[:], av_ps[:])
            nc.sync.dma_start(avb1_d[:, sl], av_sb[:])
            sm_sb = ev.tile([1, SQT], F32, name=f"b1smsb{j}", tag="evs")
            nc.vector.tensor_copy(sm_sb[:], sm_ps[:])
            nc.sync.dma_start(sumsb1_d[:, sl], sm_sb[:])

        # ---- phase B2: global queries vs non-global keys ----
        av2_ps = psav.tile([P, G], F32, name="b2av", tag="psav")
        sm2_ps = pssum.tile([1, G], F32, name="b2sm", tag="pssum")
        for i in range(nblk):
            s_ps = ps.tile([P, G], F32, name=f"b2s{i}", tag="ps")
            nc.tensor.matmul(
                s_ps[:],
                lhsT=KT[:, i * P : (i + 1) * P],
                rhs=QG[:],
                start=True,
                stop=True,
            )
            p_sb = pp.tile([P, G], DT, name=f"b2p{i}", tag="pp")
            nc.scalar.activation(p_sb[:], s_ps[:], AFT.Exp, scale=scale)
            nc.vector.tensor_mul(p_sb[:], p_sb[:], mb2_t(i))
            nc.tensor.matmul(
                av2_ps[:],
                lhsT=V[:, i * P : (i + 1) * P],
                rhs=p_sb[:],
                start=(i == 0),
                stop=(i == nblk - 1),
            )
            nc.tensor.matmul(
                sm2_ps[:],
                lhsT=ones,
                rhs=p_sb[:],
                start=(i == 0),
                stop=(i == nblk - 1),
            )
        av2_sb = ev.tile([P, G], F32, name="b2avsb", tag="ev")
        nc.vector.tensor_copy(av2_sb[:], av2_ps[:])
        nc.sync.dma_start(avb2_d[:], av2_sb[:])
        sm2_sb = ev.tile([1, G], F32, name="b2smsb", tag="evs")
        nc.vector.tensor_copy(sm2_sb[:], sm2_ps[:])
        nc.sync.dma_start(sumsb2_d[:], sm2_sb[:])

    nc.compile()
    return nc


def _in_maps(q, k, v, Wq, bq, Wk, bk, Wv, bv, S):
    _, _, mb1, _ = _host_masks(S)
    shared = {
        "cc": _pack_consts(Wq, Wk, Wv, S),
        "biases": np.ascontiguousarray(
            np.stack([bq, bk, bv], axis=1).astype(np.float32)
        ),
        "mb1": mb1,
    }
    maps = []
    for b in range(q.shape[0]):
        m = dict(shared)
        m["qt"] = np.ascontiguousarray(q[b].T)
        m["kt"] = np.ascontiguousarray(k[b].T)
        m["vt"] = np.ascontiguousarray(v[b].T)
        maps.append(m)
    return maps


def _assemble(results, S):
    gtok = _gtok(S)
    nb = len(results)
    out = np.empty((nb, S, P), dtype=np.float32)
    for b, r in enumerate(results):
        avt = r["avt"] + r["avb1"]
        sums = (r["sums"] + r["sumsb1"])[0]
        avt[:, gtok] += r["avb2"]
        sums[gtok] += r["sumsb2"][0]
        out[b] = (avt / sums[None, :]).T
    return out


_NC_CACHE = {}


def kernel(q, k, v, Wq, bq, Wk, bk, Wv, bv):
    from concourse.bass_utils import run_bass_kernel_spmd

    q = np.asarray(q, dtype=np.float32)
    k = np.asarray(k, dtype=np.float32)
    v = np.asarray(v, dtype=np.float32)
    S = q.shape[1]
    if S not in _NC_CACHE:
        _NC_CACHE[S] = build_nc(S=S)
    nc = _NC_CACHE[S]
    maps = _in_maps(
        q, k, v,
        np.asarray(Wq, np.float32), np.asarray(bq, np.float32),
        np.asarray(Wk, np.float32), np.asarray(bk, np.float32),
        np.asarray(Wv, np.float32), np.asarray(bv, np.float32),
        S,
    )
    res = run_bass_kernel_spmd(nc, maps, core_ids=list(range(len(maps))))
    return _assemble(res.results, S)
